# revision 51
# baseline (speedup 1.0000x reference)
"""Trainium2 Bass kernel for nn_Loss_65781719105930 (YOLO-style detection loss).

Strategy (pure data parallelism, 8 cores, 32 images each):
  host:   replicate the reference's target-build scatter (small int64 inputs),
          compact occupied cells, pre-pack aux tables + prediction columns into
          four contiguous DMA payloads; gather the target-class logit per
          (cell, anchor) host-side.
  device: dense pass over the 5 conf channels (sum of sigmoid^2), plus IoU /
          first-argmax / best-anchor-select / cross-entropy on compacted tiles.

Numeric tricks that keep the scalar engine on ONE activation-table set
(exp_and_others = {tanh, exp, square}):
  sigmoid(x)   = (1 + tanh(x/2)) / 2      -> work in xi = 2x-1 coords; the
                                             0.5 factors fold into host consts
  sqrt(exp(x)*anchor) = exp(x/2)*sqrt(anchor)
  ln(x)        ~ bitcast_i16(bf16 x) * ln2/2^7 - 126.94269504*ln2
                 (mean-centered log2 bit trick; loss tolerance is 2e-2 rel)

The grid offset cancels algebraically in both the IoU and the box loss, so it
never appears on device.

Device program layout: SRC [P, 7*T*A] holds quantities q = (w, h, x, y, u, uu,
ce) per (cell t, anchor a); one mul by fmask + one reduce selects the best
anchor for all seven; one final grouped reduce produces all loss partial sums
at once (per-column scales and the obj-count corrections applied on host).
"""
import numpy as np

# ---------------------------------------------------------------- constants
NCLS = 20
H = W = 32
HWC = H * W            # 1024 cells/image
A = 5
M = 50
B = 256
CORES = 8
BC = B // CORES        # 32 images per core
CH = A * (5 + NCLS)    # 125 channels
P = 128
LAM_COORD, LAM_OBJ, LAM_NOOBJ, LAM_CLS = 5.0, 1.0, 0.5, 1.0

LN2 = float(np.log(2.0))
LOG_BIAS = 126.94269504   # mean-centering constant for the log2 bit trick

_CACHE = {}


def _bf16(x):
    """float32 ndarray -> ml_dtypes.bfloat16 (RNE)."""
    import ml_dtypes
    return np.asarray(x, dtype=np.float32).astype(ml_dtypes.bfloat16)


# ---------------------------------------------------------------- host prep
def _build_target_np(gt_boxes, gt_classes, num_box):
    """Numpy replication of reference.build_target (last object wins, first-max
    class argmax). Returns per-cell [B, HWC] arrays."""
    Bn = gt_boxes.shape[0]
    valid = np.arange(M)[None, :] < num_box[:, None]
    x = gt_boxes[..., 0].astype(np.float32) * H
    y = gt_boxes[..., 1].astype(np.float32) * H
    gx = np.floor(x).astype(np.int64)
    gy = np.floor(y).astype(np.int64)
    flat = np.where(valid, gy * W + gx, HWC)
    bi = np.broadcast_to(np.arange(Bn)[:, None], (Bn, M))

    vals = np.stack([np.ones_like(x), x - gx, y - gy,
                     gt_boxes[..., 2].astype(np.float32) * H,
                     gt_boxes[..., 3].astype(np.float32) * H], axis=-1)
    tgt_box = np.zeros((Bn, HWC + 1, 5), dtype=np.float32)
    tgt_box[bi, flat] = vals
    tgt_cls = np.zeros((Bn, HWC + 1, NCLS), dtype=np.float32)
    tgt_cls[bi, flat, gt_classes.astype(np.int64)] = 1.0

    tgt_box = tgt_box[:, :HWC]
    obj = tgt_box[..., 0]
    cls_t = np.argmax(tgt_cls[:, :HWC], axis=-1).astype(np.int64)
    return obj, tgt_box[..., 1], tgt_box[..., 2], tgt_box[..., 3], tgt_box[..., 4], cls_t


def _split_multi_waits(nc):
    """This container's walrus accepts only ONE sem-wait per instruction; hoist
    extra waits onto standalone NoOps."""
    import concourse.mybir as mybir
    import bass_rust
    n = 0
    for fn in nc.m.functions:
        for blk in fn.blocks:
            new = []
            for ins in blk.instructions:
                si = ins.sync_info
                waits = list(si.on_wait) if si is not None else []
                if len(waits) > 1:
                    for w in waits[:-1]:
                        nop = mybir.InstNoOp(name=f"{ins.name}-w{n}")
                        nop.engine = ins.engine
                        nop.sync_info = bass_rust.SyncInfo(on_wait=[w], on_update=[])
                        new.append(nop)
                        n += 1
                    si.on_wait = [waits[-1]]
                    ins.sync_info = si
                new.append(ins)
            blk.instructions = new
    return n


def _offsets(T):
    """fpackA/fpackB free-dim offsets. cols_xw chan order is (x, y, conf, w, h).
    fpackA carries everything the IoU chain needs (DMA'd first); fpackB the
    late aux tables (selection/loss stage)."""
    o = {}
    o["XW"] = 0
    o["B1"] = 25 * T
    o["B2"] = o["B1"] + 2 * T
    o["TAREA"] = o["B2"] + 2 * T
    o["WCONST"] = o["TAREA"] + T
    o["SQA"] = o["WCONST"] + 5 * T
    o["NFA"] = o["SQA"] + 10
    o["SAUX"] = 0
    o["AUX4"] = 5 * T
    o["NFB"] = o["AUX4"] + 4 * T
    return o


# ---------------------------------------------------------------- bass build
def _build_nc(T, split=True):
    """Build the per-core kernel for T cell-blocks per partition (P*T slots).

    DMA payloads (issue order: fpackA, lgpack, confd, fpackB):
    fpackA [P, NFA] f32 -- everything the IoU/argmax chain reads:
      cols_xw (t, ch{x,y,conf,w,h}, a)  25T
      B1, B2  (d{x,y}, t)               2T each   xi-space target box edges
      TAREA   (t)                       T         tw*th (physical, cell units)
      WCONST  (t, a)                    5T        argmax tiebreak: (A - a) for
                                                  real cells, [1,0,0,0,0] for
                                                  padding (one-hot anchor 0)
      SQA     (d{w,h}, a)               10        sqrt(anchor)
    lgpack bf16 [P, 100T]: logits (t, a, j)
    confd  bf16 [P, 1280]: all conf channels (dense noobj pass)
    fpackB [P, NFB] f32 -- late aux for the selection/loss stage:
      S_AUX   (t, a)                    5T        target-class logit (padding:
                                                  exact device lse of 20.0)
      AUX4    (q{w,h,x,y}, t)           4T        (sqrt tw, sqrt th, 2xo-1,
                                                  2yo-1); padding rows hold the
                                                  padded prediction values so
                                                  their sq-diffs vanish
    partials out [P, 12] (host applies per-column scales):
      0..3 box (w,h,x,y) sq-diff sums, 4 sum u_sel, 5 sum u_sel^2,
      6 sum ce_sel, 7 dense sum ud, 8 dense sum ud^2  (u/ud = tanh(conf/2))
    """
    import concourse.bass as bass
    import concourse.mybir as mybir
    import concourse.tile as tile

    f32 = mybir.dt.float32
    bf16 = mybir.dt.bfloat16
    i32 = mybir.dt.int32
    AF = mybir.ActivationFunctionType
    OP = mybir.AluOpType
    AX = mybir.AxisListType

    TA = T * A
    TA2 = TA * 2
    O = _offsets(T)
    NFA, NFB = O["NFA"], O["NFB"]
    DF = BC * A * HWC // P   # 1280 dense conf elements per partition

    def _v(ap, off, dims):
        """Sub-view of a tile AP: keep its partition dim, replace free dims."""
        return bass.AP(tensor=ap.tensor, offset=ap.offset + off,
                       ap=[list(ap.ap[0])] + dims)

    # Suppress the Bass-init all-engine barrier (it only orders the const-AP
    # memsets, whose single consumer -- the activation bias read -- runs ~4us
    # after them on this kernel's timeline). Saves ~1-2us of prologue.
    import os as _os
    _patch = _os.environ.get("K_KEEP_INIT_BARRIER", "0") != "1"
    _orig_barrier = bass.Bass.all_engine_barrier
    if _patch:
        bass.Bass.all_engine_barrier = lambda self, **kw: None
    try:
        nc = bass.Bass("TRN2")
    finally:
        if _patch:
            bass.Bass.all_engine_barrier = _orig_barrier
    fpackA_d = nc.declare_dram_parameter("fpackA", [P, NFA], f32, isOutput=False)
    fpackB_d = nc.declare_dram_parameter("fpackB", [P, NFB], f32, isOutput=False)
    lgpack_d = nc.declare_dram_parameter("lgpack", [P, 100 * T], bf16, isOutput=False)
    confd_d = nc.declare_dram_parameter("confd", [P, DF], bf16, isOutput=False)
    partials_d = nc.declare_dram_parameter("partials", [P, 12], f32, isOutput=True)

    with tile.TileContext(nc) as tc:
        with tc.tile_pool(name="sb", bufs=1) as pool:
            # ---------------- input DMAs, priority order, on sync HWDGE
            fp = pool.tile([P, NFA], f32, name="fp")
            nc.sync.dma_start(out=fp[:], in_=fpackA_d[:])
            lg_in = pool.tile([P, 100 * T], bf16, name="lg_in")
            nc.sync.dma_start(out=lg_in[:], in_=lgpack_d[:])
            confd = pool.tile([P, DF], bf16, name="confd")
            nc.sync.dma_start(out=confd[:], in_=confd_d[:])
            fpb = pool.tile([P, NFB], f32, name="fpb")
            nc.sync.dma_start(out=fpb[:], in_=fpackB_d[:])

            partials = pool.tile([P, 12], f32, name="partials")

            # SRC: (q, t, a) with q in {w, h, x, y, u, uu, ce}
            SRC = pool.tile([P, 7 * TA], f32, name="SRC")

            # ---------------- scalar engine program (one act-table set)
            # EW = exp(chan{w,h}/2): chans 3,4 of cols_xw -> (t, a, d)
            EW = pool.tile([P, TA2], f32, name="EW")
            nc.scalar.activation(
                _v(EW[:], 0, [[1, 2], [2 * A, T], [2, A]]),
                _v(fp[:], O["XW"] + 15, [[5, 2], [25, T], [1, A]]),
                AF.Exp, scale=0.5)
            # x, y, u = tanh(chan{x,y,conf}/2) -> SRC q2, q3, q4
            nc.scalar.activation(
                _v(SRC[:], 2 * TA, [[TA, 3], [A, T], [1, A]]),
                _v(fp[:], O["XW"], [[5, 3], [25, T], [1, A]]),
                AF.Tanh, scale=0.5)
            # e = exp(logits), bf16, (t, a, j)
            e = pool.tile([P, 100 * T], bf16, name="e")
            nc.scalar.activation(
                _v(e[:], 0, [[100, T], [NCLS, A], [1, NCLS]]),
                _v(lg_in[:], 0, [[100, T], [NCLS, A], [1, NCLS]]),
                AF.Exp)
            # uu = u^2 -> SRC q5 (square is in the same act-table set)
            nc.scalar.activation(_v(SRC[:], 5 * TA, [[1, TA]]),
                                 _v(SRC[:], 4 * TA, [[1, TA]]), AF.Square)
            # dense noobj pass: ud = tanh(c/2); sum sigma^2 = 0.25*(N + 2*sum ud
            # + sum ud^2)
            UD = pool.tile([P, DF], f32, name="UD")
            nc.scalar.activation(UD[:], confd[:], AF.Tanh, scale=0.5,
                                 accum_out=_v(partials[:], 7, [[1, 1]]))
            SQD = pool.tile([P, DF], f32, name="SQD")
            nc.scalar.activation(SQD[:], UD[:], AF.Square,
                                 accum_out=_v(partials[:], 8, [[1, 1]]))

            # ---------------- vector engine program
            tcnt = [0]

            def tmp(n, dtype=f32):
                tcnt[0] += 1
                return pool.tile([P, n], dtype, name=f"t{tcnt[0]}")

            # sh = EW * sqrt(anchor) = sqrt(pred_wh) -> SRC q0, q1
            SH = _v(SRC[:], 0, [[TA, 2], [A, T], [1, A]])
            nc.vector.tensor_tensor(
                out=SH,
                in0=_v(EW[:], 0, [[1, 2], [2 * A, T], [2, A]]),
                in1=_v(fp[:], O["SQA"], [[1, 2], [0, T], [2, A]]),
                op=OP.mult)
            # wfull = sh*sh = pred_wh (xi-space half-width), (d, t, a)
            wf = tmp(TA2)
            SH2 = _v(SRC[:], 0, [[TA, 2], [1, TA]])
            WF = _v(wf[:], 0, [[TA, 2], [1, TA]])
            nc.vector.tensor_tensor(out=WF, in0=SH2, in1=SH2, op=OP.mult)

            # IoU in xi coords. XY = SRC q2, q3 as (d, t, a)
            XY = _v(SRC[:], 2 * TA, [[TA, 2], [1, TA]])
            lo = tmp(TA2)
            nc.vector.tensor_tensor(out=lo[:], in0=XY, in1=WF, op=OP.subtract)
            hi = tmp(TA2)
            nc.vector.tensor_tensor(out=hi[:], in0=XY, in1=WF, op=OP.add)
            B1v = _v(fp[:], O["B1"], [[T, 2], [1, T], [0, A]])
            B2v = _v(fp[:], O["B2"], [[T, 2], [1, T], [0, A]])
            LOv = _v(lo[:], 0, [[TA, 2], [A, T], [1, A]])
            HIv = _v(hi[:], 0, [[TA, 2], [A, T], [1, A]])
            t1 = tmp(TA2)
            nc.vector.tensor_tensor(out=_v(t1[:], 0, [[TA, 2], [A, T], [1, A]]),
                                    in0=HIv, in1=B2v, op=OP.min)
            t2 = tmp(TA2)
            nc.vector.tensor_tensor(out=_v(t2[:], 0, [[TA, 2], [A, T], [1, A]]),
                                    in0=LOv, in1=B1v, op=OP.max)
            t3 = tmp(TA2)
            nc.vector.tensor_tensor(out=t3[:], in0=t1[:], in1=t2[:], op=OP.subtract)
            # iw = max(t3, 0) -> xi-space overlap widths (2x physical); the
            # union side is scaled x4 to stay consistent (iou is scale-free)
            iwih = tmp(TA2)
            nc.vector.tensor_scalar_max(iwih[:], t3[:], 0.0)
            # cls path: se = sum_j e via a split adder tree. The two big
            # levels run as bf16 TT adds on vector (2x-eligible, unlike
            # tensor_reduce); the three small levels + lse + ce hide on gpsimd.
            s1 = tmp(10 * TA, bf16)
            nc.vector.tensor_tensor(out=_v(s1[:], 0, [[10, TA], [1, 10]]),
                                    in0=_v(e[:], 0, [[NCLS, TA], [1, 10]]),
                                    in1=_v(e[:], 10, [[NCLS, TA], [1, 10]]),
                                    op=OP.add)
            s2 = tmp(5 * TA, bf16)
            nc.vector.tensor_tensor(out=_v(s2[:], 0, [[5, TA], [1, 5]]),
                                    in0=_v(s1[:], 0, [[10, TA], [1, 5]]),
                                    in1=_v(s1[:], 5, [[10, TA], [1, 5]]),
                                    op=OP.add)
            inter = tmp(TA)
            nc.vector.tensor_tensor(out=inter[:], in0=_v(iwih[:], 0, [[1, TA]]),
                                    in1=_v(iwih[:], TA, [[1, TA]]), op=OP.mult)
            # union side branch on gpsimd, in parallel with the inter chain
            # (4*areaA as (2w)(2h): gpsimd tensor_scalar is ~2.5x slower than
            # its tensor_tensor, so double with an add instead)
            wf2 = tmp(TA2)
            nc.gpsimd.tensor_tensor(out=wf2[:], in0=wf[:], in1=wf[:], op=OP.add)
            areaA4 = tmp(TA)
            nc.gpsimd.tensor_tensor(out=areaA4[:], in0=_v(wf2[:], 0, [[1, TA]]),
                                    in1=_v(wf2[:], TA, [[1, TA]]), op=OP.mult)
            u1 = tmp(TA)
            nc.gpsimd.tensor_tensor(out=_v(u1[:], 0, [[A, T], [1, A]]),
                                    in0=_v(areaA4[:], 0, [[A, T], [1, A]]),
                                    in1=_v(fp[:], O["TAREA"], [[1, T], [0, A]]),
                                    op=OP.add)
            u2 = tmp(TA)
            nc.vector.tensor_tensor(out=u2[:], in0=u1[:], in1=inter[:],
                                    op=OP.subtract)
            rcp = tmp(TA)
            nc.vector.reciprocal(out=rcp[:], in_=u2[:])
            iou = tmp(TA)
            nc.vector.tensor_tensor(out=iou[:], in0=inter[:], in1=rcp[:],
                                    op=OP.mult)

            # first-argmax over a via bit-packed keys: clear the low 3
            # mantissa bits of iou (all ious >= 0, so int order == float
            # order) and OR in a per-anchor tiebreak (7 - a). Ties -- exact
            # at 8-ULP granularity -- resolve to the FIRST anchor, matching
            # jnp.argmax. Padding cells (iou all +0) one-hot-select anchor 0.
            ipk = tmp(TA, i32)
            nc.vector.tensor_scalar(out=ipk[:], in0=iou[:].bitcast(i32),
                                    scalar1=-8, scalar2=None,
                                    op0=OP.bitwise_and)
            ipk2 = tmp(TA, i32)
            nc.vector.tensor_tensor(out=_v(ipk2[:], 0, [[A, T], [1, A]]),
                                    in0=_v(ipk[:], 0, [[A, T], [1, A]]),
                                    in1=_v(fp[:], O["WCONST"],
                                           [[0, T], [1, A]]).bitcast(i32),
                                    op=OP.bitwise_or)
            rmax = tmp(T, i32)
            nc.vector.tensor_reduce(out=rmax[:],
                                    in_=_v(ipk2[:], 0, [[A, T], [1, A]]),
                                    axis=AX.X, op=OP.max)
            fmask = tmp(TA)
            nc.vector.tensor_tensor(out=_v(fmask[:], 0, [[A, T], [1, A]]),
                                    in0=_v(ipk2[:], 0, [[A, T], [1, A]]),
                                    in1=_v(rmax[:], 0, [[1, T], [0, A]]),
                                    op=OP.is_equal)
            s3 = tmp(2 * TA)
            nc.gpsimd.tensor_tensor(out=_v(s3[:], 0, [[2, TA], [1, 2]]),
                                    in0=_v(s2[:], 0, [[5, TA], [1, 2]]),
                                    in1=_v(s2[:], 2, [[5, TA], [1, 2]]),
                                    op=OP.add)
            s4 = tmp(TA)
            nc.gpsimd.tensor_tensor(out=_v(s4[:], 0, [[1, TA]]),
                                    in0=_v(s3[:], 0, [[2, TA]]),
                                    in1=_v(s3[:], 1, [[2, TA]]),
                                    op=OP.add)
            se = tmp(TA)
            nc.gpsimd.tensor_tensor(out=_v(se[:], 0, [[1, TA]]),
                                    in0=_v(s4[:], 0, [[1, TA]]),
                                    in1=_v(s2[:], 4, [[5, TA]]),
                                    op=OP.add)
            lgf = tmp(TA)
            nc.gpsimd.tensor_copy(out=lgf[:], in_=se[:].bitcast(i32))
            lse = tmp(TA)
            nc.gpsimd.tensor_scalar(out=lse[:], in0=lgf[:],
                                    scalar1=LN2 / (1 << 23),
                                    scalar2=-LOG_BIAS * LN2,
                                    op0=OP.mult, op1=OP.add)
            nc.gpsimd.tensor_tensor(out=_v(SRC[:], 6 * TA, [[A, T], [1, A]]),
                                    in0=_v(lse[:], 0, [[A, T], [1, A]]),
                                    in1=_v(fpb[:], O["SAUX"], [[A, T], [1, A]]),
                                    op=OP.subtract)

            # best-anchor selection of all seven quantities in one mul+reduce
            selm = pool.tile([P, 7 * TA], f32, name="selm")
            nc.vector.tensor_tensor(out=_v(selm[:], 0, [[TA, 7], [1, TA]]),
                                    in0=_v(SRC[:], 0, [[TA, 7], [1, TA]]),
                                    in1=_v(fmask[:], 0, [[0, 7], [1, TA]]),
                                    op=OP.mult)
            FIN = pool.tile([P, 7 * T], f32, name="FIN")
            nc.vector.tensor_reduce(out=_v(FIN[:], 0, [[T, 7], [1, T]]),
                                    in_=_v(selm[:], 0, [[TA, 7], [A, T], [1, A]]),
                                    axis=AX.X, op=OP.add)

            # box: FIN q0..3 -> oscl4*(sel - aux4)^2, written back into q0..3
            d4 = tmp(4 * T)
            nc.vector.tensor_tensor(out=d4[:], in0=_v(FIN[:], 0, [[1, 4 * T]]),
                                    in1=_v(fpb[:], O["AUX4"], [[1, 4 * T]]),
                                    op=OP.subtract)
            nc.vector.tensor_tensor(out=_v(FIN[:], 0, [[1, 4 * T]]),
                                    in0=d4[:], in1=d4[:], op=OP.mult)

            # one grouped reduce -> partials cols 0..6
            nc.vector.tensor_reduce(out=_v(partials[:], 0, [[1, 7]]),
                                    in_=_v(FIN[:], 0, [[T, 7], [1, T]]),
                                    axis=AX.X, op=OP.add)

            nc.sync.dma_start(out=partials_d[:], in_=partials[:])

    if split:
        _split_multi_waits(nc)
    return nc


# -------------------------------------------------------------- shard builder
def _make_in_maps(out, gt_boxes, anchor_np, gt_classes_np, num_box_np, T):
    obj, xo, yo, tw, th, cls_t = _build_target_np(gt_boxes, gt_classes_np,
                                                  num_box_np)
    SLOTS = P * T
    TA = T * A
    out_r = out.reshape(B, A, 25, HWC)
    sqa = np.sqrt(anchor_np)                       # [A, 2]

    in_maps = []
    for c in range(CORES):
        sl = slice(c * BC, (c + 1) * BC)
        ob = obj[sl]                               # [BC, HWC]
        bloc, hwloc = np.nonzero(ob > 0)
        K = len(bloc)
        assert K <= SLOTS

        def place(vals):
            buf = np.zeros(SLOTS, dtype=np.float32)
            buf[:K] = vals
            return buf.reshape(P, T)

        objv = place(np.ones(K, dtype=np.float32))
        xov = place(xo[sl][bloc, hwloc])
        yov = place(yo[sl][bloc, hwloc])
        twv = place(tw[sl][bloc, hwloc])
        thv = place(th[sl][bloc, hwloc])

        # occupied-cell prediction channels [K, A, 25]
        colsb = np.zeros((SLOTS, A, 25), dtype=np.float32)
        if K:
            colsb[:K] = out_r[sl].transpose(0, 3, 1, 2)[bloc, hwloc]
        # cols_xw (t, ch{x,y,conf,w,h}, a)
        sel = colsb[:, :, [21, 22, 20, 23, 24]]            # (t, a, ch)
        cols_xw = np.ascontiguousarray(
            sel.reshape(P, T, A, 5).transpose(0, 1, 3, 2)).reshape(P, 25 * T)
        logits = np.ascontiguousarray(
            colsb[:, :, :20]).reshape(P, 100 * T)          # (t, a, j)

        # target-class logit per (t, a); padding slots get the exact device
        # lse of se=20.0 (logits 0 -> e all 1) so padded ce comes out 0
        clsv = place(cls_t[sl][bloc, hwloc].astype(np.float32)).astype(np.int64)
        s_aux = np.take_along_axis(
            colsb[:, :, :20].reshape(SLOTS, A, 20),
            clsv.reshape(SLOTS, 1, 1).repeat(A, axis=1), axis=2
        )[:, :, 0].astype(np.float32)
        i32pad = float(np.float32(20.0).view(np.int32))
        lse_pad = np.float32(np.float32(i32pad * np.float32(LN2 / (1 << 23)))
                             + np.float32(-LOG_BIAS * LN2))
        s_aux[K:] = lse_pad
        s_aux = s_aux.reshape(P, TA)

        # xi-space target box edges (d{x,y}, t): center 2o-1, half-width t_wh
        cxv = 2.0 * xov - 1.0
        cyv = 2.0 * yov - 1.0
        b1 = np.stack([cxv - twv, cyv - thv], axis=1).reshape(P, 2 * T)
        b2 = np.stack([cxv + twv, cyv + thv], axis=1).reshape(P, 2 * T)
        tarea = (4.0 * twv * thv).reshape(P, T)

        # argmax tiebreak bits (7 - a) as raw int32, shipped bitcast as f32
        wconst = np.broadcast_to(
            (7 - np.arange(A, dtype=np.int32)).view(np.float32), (P, A))
        wconst = np.concatenate(
            [wconst, np.zeros((P, 5 * T - A), dtype=np.float32)], axis=1)
        sqav = np.broadcast_to(sqa.reshape(1, 10), (P, 10))

        # AUX4 in q-order (w, h, x, y); padding rows get the exact padded
        # prediction (sqrt(anchor0), tanh(0)=0) so their sq-diffs vanish
        aux4 = np.stack([np.sqrt(twv), np.sqrt(thv), cxv, cyv], axis=1)
        a4f = aux4.transpose(0, 2, 1).reshape(SLOTS, 4)
        a4f[K:] = [float(sqa[0, 0]), float(sqa[0, 1]), 0.0, 0.0]
        aux4 = np.ascontiguousarray(
            a4f.reshape(P, T, 4).transpose(0, 2, 1)).reshape(P, 4 * T)

        fpackA = np.concatenate(
            [cols_xw, b1, b2, tarea, wconst, sqav], axis=1)
        fpackB = np.concatenate([s_aux, aux4], axis=1)

        # dense conf channels: [BC, A, HWC] -> [P, 1280] bf16
        confd = out_r[sl][:, :, 20, :].reshape(P, -1)

        in_maps.append({
            "fpackA": np.ascontiguousarray(fpackA, dtype=np.float32),
            "fpackB": np.ascontiguousarray(fpackB, dtype=np.float32),
            "lgpack": _bf16(logits),
            "confd": _bf16(confd),
        })
    return in_maps


# ---------------------------------------------------------------- entry point
def kernel(out, gt_boxes, anchor, gt_classes, num_box):
    from concourse.bass_utils import run_bass_kernel_spmd

    out = np.ascontiguousarray(np.asarray(out, dtype=np.float32))
    gt_boxes = np.asarray(gt_boxes, dtype=np.float32)
    anchor_np = np.asarray(anchor, dtype=np.float32)
    gt_classes_np = np.asarray(gt_classes)
    num_box_np = np.asarray(num_box)

    # per-core occupied-cell counts decide the compiled tile factor T
    obj = _build_target_np(gt_boxes, gt_classes_np, num_box_np)[0]
    ks = [int((obj[c * BC:(c + 1) * BC] > 0).sum()) for c in range(CORES)]
    maxk = max(ks)
    T = max(1, -(-maxk // P))
    assert maxk <= 13 * P and T <= 13

    in_maps = _make_in_maps(out, gt_boxes, anchor_np, gt_classes_np,
                            num_box_np, T)

    import os
    key = f"nc{T}"
    if key not in _CACHE:
        _CACHE[key] = _build_nc(T)
    trace = os.environ.get("KERNEL_TRACE", "0") == "1"
    res = None
    for attempt in range(3):
        try:
            res = run_bass_kernel_spmd(_CACHE[key], in_maps,
                                       core_ids=list(range(CORES)), trace=trace)
            break
        except Exception:
            # transient device/runtime hiccups (e.g. NRT_EXEC_UNIT_UNRECOVERABLE)
            # recover on retry; re-raise only if persistent
            if attempt == 2:
                raise
            import time
            time.sleep(2.0)
    if trace:
        print(f"HW exec time: {res.exec_time_ns} ns  (mean {res.mean_exec_time_ns})")

    cols = np.zeros(12, dtype=np.float64)
    for c in range(CORES):
        cols += res.results[c]["partials"].astype(np.float64).sum(axis=0)
    K = float(sum(ks))
    box_loss = np.float32(LAM_COORD / B * (cols[0] + cols[1]
                                            + 0.25 * (cols[2] + cols[3])))
    conf_loss = np.float32(LAM_OBJ / B * 0.25 * (cols[5] - 2.0 * cols[4] + K))
    nob_c = 0.25 * (cols[5] + 2.0 * cols[4] + K)
    dense = 0.25 * (float(B * A * HWC) + 2.0 * cols[7] + cols[8])
    noobj_loss = np.float32(LAM_NOOBJ / B * (dense - nob_c))
    cls_loss = np.float32(LAM_CLS / B * cols[6])
    return (box_loss, conf_loss, noobj_loss, cls_loss)


# revision 52
# speedup vs baseline: 1.1886x; 1.1886x over previous
"""Trainium2 Bass kernel for nn_Loss_65781719105930 (YOLO-style detection loss).

Strategy (pure data parallelism, 8 cores, 32 images each):
  host:   replicate the reference's target-build scatter (small int64 inputs),
          compact occupied cells, pre-pack aux tables + prediction columns into
          four contiguous DMA payloads; gather the target-class logit per
          (cell, anchor) host-side.
  device: dense pass over the 5 conf channels (sum of sigmoid^2), plus IoU /
          first-argmax / best-anchor-select / cross-entropy on compacted tiles.

Numeric tricks that keep the scalar engine on ONE activation-table set
(exp_and_others = {tanh, exp, square}):
  sigmoid(x)   = (1 + tanh(x/2)) / 2      -> work in xi = 2x-1 coords; the
                                             0.5 factors fold into host consts
  sqrt(exp(x)*anchor) = exp(x/2)*sqrt(anchor)
  ln(x)        ~ bitcast_i16(bf16 x) * ln2/2^7 - 126.94269504*ln2
                 (mean-centered log2 bit trick; loss tolerance is 2e-2 rel)

The grid offset cancels algebraically in both the IoU and the box loss, so it
never appears on device.

Device program layout: SRC [P, 7*T*A] holds quantities q = (w, h, x, y, u, uu,
ce) per (cell t, anchor a); one mul by fmask + one reduce selects the best
anchor for all seven; one final grouped reduce produces all loss partial sums
at once (per-column scales and the obj-count corrections applied on host).
"""
import numpy as np

# ---------------------------------------------------------------- constants
NCLS = 20
H = W = 32
HWC = H * W            # 1024 cells/image
A = 5
M = 50
B = 256
CORES = 8
BC = B // CORES        # 32 images per core
CH = A * (5 + NCLS)    # 125 channels
P = 128
LAM_COORD, LAM_OBJ, LAM_NOOBJ, LAM_CLS = 5.0, 1.0, 0.5, 1.0

LN2 = float(np.log(2.0))
LOG_BIAS = 126.94269504   # mean-centering constant for the log2 bit trick

_CACHE = {}


def _bf16(x):
    """float32 ndarray -> ml_dtypes.bfloat16 (RNE)."""
    import ml_dtypes
    return np.asarray(x, dtype=np.float32).astype(ml_dtypes.bfloat16)


# ---------------------------------------------------------------- host prep
def _build_target_np(gt_boxes, gt_classes, num_box):
    """Numpy replication of reference.build_target (last object wins, first-max
    class argmax). Returns per-cell [B, HWC] arrays."""
    Bn = gt_boxes.shape[0]
    valid = np.arange(M)[None, :] < num_box[:, None]
    x = gt_boxes[..., 0].astype(np.float32) * H
    y = gt_boxes[..., 1].astype(np.float32) * H
    gx = np.floor(x).astype(np.int64)
    gy = np.floor(y).astype(np.int64)
    flat = np.where(valid, gy * W + gx, HWC)
    bi = np.broadcast_to(np.arange(Bn)[:, None], (Bn, M))

    vals = np.stack([np.ones_like(x), x - gx, y - gy,
                     gt_boxes[..., 2].astype(np.float32) * H,
                     gt_boxes[..., 3].astype(np.float32) * H], axis=-1)
    tgt_box = np.zeros((Bn, HWC + 1, 5), dtype=np.float32)
    tgt_box[bi, flat] = vals
    tgt_cls = np.zeros((Bn, HWC + 1, NCLS), dtype=np.float32)
    tgt_cls[bi, flat, gt_classes.astype(np.int64)] = 1.0

    tgt_box = tgt_box[:, :HWC]
    obj = tgt_box[..., 0]
    cls_t = np.argmax(tgt_cls[:, :HWC], axis=-1).astype(np.int64)
    return obj, tgt_box[..., 1], tgt_box[..., 2], tgt_box[..., 3], tgt_box[..., 4], cls_t


def _split_multi_waits(nc):
    """This container's walrus accepts only ONE sem-wait per instruction; hoist
    extra waits onto standalone NoOps."""
    import concourse.mybir as mybir
    import bass_rust
    n = 0
    for fn in nc.m.functions:
        for blk in fn.blocks:
            new = []
            for ins in blk.instructions:
                si = ins.sync_info
                waits = list(si.on_wait) if si is not None else []
                if len(waits) > 1:
                    for w in waits[:-1]:
                        nop = mybir.InstNoOp(name=f"{ins.name}-w{n}")
                        nop.engine = ins.engine
                        nop.sync_info = bass_rust.SyncInfo(on_wait=[w], on_update=[])
                        new.append(nop)
                        n += 1
                    si.on_wait = [waits[-1]]
                    ins.sync_info = si
                new.append(ins)
            blk.instructions = new
    return n


def _offsets(T):
    """fpackA/fpackB free-dim offsets. cols_xw chan order is (x, y, conf, w, h).
    fpackA carries everything the IoU chain needs (DMA'd first); fpackB the
    late aux tables (selection/loss stage)."""
    o = {}
    o["XW"] = 0
    o["B1"] = 25 * T
    o["B2"] = o["B1"] + 2 * T
    o["TAREA"] = o["B2"] + 2 * T
    o["WCONST"] = o["TAREA"] + T
    o["SQA"] = o["WCONST"] + 5 * T
    o["NFA"] = o["SQA"] + 10
    o["SAUX"] = 0
    o["AUX4"] = 5 * T
    o["NFB"] = o["AUX4"] + 4 * T
    return o


# ---------------------------------------------------------------- bass build
def _build_nc(T, split=True):
    """Build the per-core kernel for T cell-blocks per partition (P*T slots).

    DMA payloads (issue order: fpackA, lgpack, confd, fpackB):
    fpackA [P, NFA] f32 -- everything the IoU/argmax chain reads:
      cols_xw (t, ch{x,y,conf,w,h}, a)  25T
      B1, B2  (d{x,y}, t)               2T each   xi-space target box edges
      TAREA   (t)                       T         tw*th (physical, cell units)
      WCONST  (t, a)                    5T        argmax tiebreak: (A - a) for
                                                  real cells, [1,0,0,0,0] for
                                                  padding (one-hot anchor 0)
      SQA     (d{w,h}, a)               10        sqrt(anchor)
    lgpack bf16 [P, 100T]: logits (t, a, j)
    confd  bf16 [P, 1280]: all conf channels (dense noobj pass)
    fpackB [P, NFB] f32 -- late aux for the selection/loss stage:
      S_AUX   (t, a)                    5T        target-class logit (padding:
                                                  exact device lse of 20.0)
      AUX4    (q{w,h,x,y}, t)           4T        (sqrt tw, sqrt th, 2xo-1,
                                                  2yo-1); padding rows hold the
                                                  padded prediction values so
                                                  their sq-diffs vanish
    partials out [P, 12] (host applies per-column scales):
      0..3 box (w,h,x,y) sq-diff sums, 4 sum u_sel, 5 sum u_sel^2,
      6 sum ce_sel, 7 dense sum ud, 8 dense sum ud^2  (u/ud = tanh(conf/2))
    """
    import concourse.bass as bass
    import concourse.mybir as mybir
    import concourse.tile as tile

    f32 = mybir.dt.float32
    bf16 = mybir.dt.bfloat16
    i32 = mybir.dt.int32
    AF = mybir.ActivationFunctionType
    OP = mybir.AluOpType
    AX = mybir.AxisListType

    TA = T * A
    TA2 = TA * 2
    O = _offsets(T)
    NFA, NFB = O["NFA"], O["NFB"]
    DF = BC * A * HWC // P   # 1280 dense conf elements per partition

    def _v(ap, off, dims):
        """Sub-view of a tile AP: keep its partition dim, replace free dims."""
        return bass.AP(tensor=ap.tensor, offset=ap.offset + off,
                       ap=[list(ap.ap[0])] + dims)

    # Suppress the Bass-init all-engine barrier (it only orders the const-AP
    # memsets, whose single consumer -- the activation bias read -- runs ~4us
    # after them on this kernel's timeline). Saves ~1-2us of prologue.
    import os as _os
    _patch = _os.environ.get("K_KEEP_INIT_BARRIER", "0") != "1"
    _orig_barrier = bass.Bass.all_engine_barrier
    if _patch:
        bass.Bass.all_engine_barrier = lambda self, **kw: None
    try:
        nc = bass.Bass("TRN2")
    finally:
        if _patch:
            bass.Bass.all_engine_barrier = _orig_barrier
    fpackA_d = nc.declare_dram_parameter("fpackA", [P, NFA], f32, isOutput=False)
    fpackB_d = nc.declare_dram_parameter("fpackB", [P, NFB], f32, isOutput=False)
    lgpack_d = nc.declare_dram_parameter("lgpack", [P, 100 * T], bf16, isOutput=False)
    confd_d = nc.declare_dram_parameter("confd", [P, DF], bf16, isOutput=False)
    partials_d = nc.declare_dram_parameter("partials", [P, 12], f32, isOutput=True)

    with tile.TileContext(nc) as tc:
        with tc.tile_pool(name="sb", bufs=1) as pool:
            # ---------------- input DMAs, priority order, on sync HWDGE
            fp = pool.tile([P, NFA], f32, name="fp")
            nc.sync.dma_start(out=fp[:], in_=fpackA_d[:])
            lg_in = pool.tile([P, 100 * T], bf16, name="lg_in")
            nc.sync.dma_start(out=lg_in[:], in_=lgpack_d[:])
            confd = pool.tile([P, DF], bf16, name="confd")
            nc.sync.dma_start(out=confd[:], in_=confd_d[:])
            fpb = pool.tile([P, NFB], f32, name="fpb")
            nc.sync.dma_start(out=fpb[:], in_=fpackB_d[:])

            partials = pool.tile([P, 12], f32, name="partials")

            # SRC: (q, t, a) with q in {w, h, x, y, u, uu, ce}
            SRC = pool.tile([P, 7 * TA], f32, name="SRC")

            # ---------------- scalar engine program (one act-table set)
            # EW = exp(chan{w,h}/2): chans 3,4 of cols_xw -> (t, a, d)
            EW = pool.tile([P, TA2], f32, name="EW")
            nc.scalar.activation(
                _v(EW[:], 0, [[1, 2], [2 * A, T], [2, A]]),
                _v(fp[:], O["XW"] + 15, [[5, 2], [25, T], [1, A]]),
                AF.Exp, scale=0.5)
            # x, y, u = tanh(chan{x,y,conf}/2) -> SRC q2, q3, q4
            nc.scalar.activation(
                _v(SRC[:], 2 * TA, [[TA, 3], [A, T], [1, A]]),
                _v(fp[:], O["XW"], [[5, 3], [25, T], [1, A]]),
                AF.Tanh, scale=0.5)
            # e = exp(logits), bf16, (t, a, j)
            e = pool.tile([P, 100 * T], bf16, name="e")
            nc.scalar.activation(
                _v(e[:], 0, [[100, T], [NCLS, A], [1, NCLS]]),
                _v(lg_in[:], 0, [[100, T], [NCLS, A], [1, NCLS]]),
                AF.Exp)
            # uu = u^2 -> SRC q5 (square is in the same act-table set)
            nc.scalar.activation(_v(SRC[:], 5 * TA, [[1, TA]]),
                                 _v(SRC[:], 4 * TA, [[1, TA]]), AF.Square)
            # dense noobj pass: ud = tanh(c/2); sum sigma^2 = 0.25*(N + 2*sum ud
            # + sum ud^2)
            UD = pool.tile([P, DF], f32, name="UD")
            nc.scalar.activation(UD[:], confd[:], AF.Tanh, scale=0.5,
                                 accum_out=_v(partials[:], 7, [[1, 1]]))
            SQD = pool.tile([P, DF], f32, name="SQD")
            nc.scalar.activation(SQD[:], UD[:], AF.Square,
                                 accum_out=_v(partials[:], 8, [[1, 1]]))

            # ---------------- vector engine program
            tcnt = [0]

            def tmp(n, dtype=f32):
                tcnt[0] += 1
                return pool.tile([P, n], dtype, name=f"t{tcnt[0]}")

            # sh = EW * sqrt(anchor) = sqrt(pred_wh) -> SRC q0, q1
            SH = _v(SRC[:], 0, [[TA, 2], [A, T], [1, A]])
            nc.vector.tensor_tensor(
                out=SH,
                in0=_v(EW[:], 0, [[1, 2], [2 * A, T], [2, A]]),
                in1=_v(fp[:], O["SQA"], [[1, 2], [0, T], [2, A]]),
                op=OP.mult)
            # wfull = sh*sh = pred_wh (xi-space half-width), (d, t, a)
            wf = tmp(TA2)
            SH2 = _v(SRC[:], 0, [[TA, 2], [1, TA]])
            WF = _v(wf[:], 0, [[TA, 2], [1, TA]])
            nc.vector.tensor_tensor(out=WF, in0=SH2, in1=SH2, op=OP.mult)

            # IoU in xi coords. XY = SRC q2, q3 as (d, t, a)
            XY = _v(SRC[:], 2 * TA, [[TA, 2], [1, TA]])
            lo = tmp(TA2)
            nc.vector.tensor_tensor(out=lo[:], in0=XY, in1=WF, op=OP.subtract)
            hi = tmp(TA2)
            nc.vector.tensor_tensor(out=hi[:], in0=XY, in1=WF, op=OP.add)
            B1v = _v(fp[:], O["B1"], [[T, 2], [1, T], [0, A]])
            B2v = _v(fp[:], O["B2"], [[T, 2], [1, T], [0, A]])
            LOv = _v(lo[:], 0, [[TA, 2], [A, T], [1, A]])
            HIv = _v(hi[:], 0, [[TA, 2], [A, T], [1, A]])
            t1 = tmp(TA2)
            nc.vector.tensor_tensor(out=_v(t1[:], 0, [[TA, 2], [A, T], [1, A]]),
                                    in0=HIv, in1=B2v, op=OP.min)
            t2 = tmp(TA2)
            nc.vector.tensor_tensor(out=_v(t2[:], 0, [[TA, 2], [A, T], [1, A]]),
                                    in0=LOv, in1=B1v, op=OP.max)
            t3 = tmp(TA2)
            nc.vector.tensor_tensor(out=t3[:], in0=t1[:], in1=t2[:], op=OP.subtract)
            # iw = max(t3, 0) -> xi-space overlap widths (2x physical); the
            # union side is scaled x4 to stay consistent (iou is scale-free)
            iwih = tmp(TA2)
            nc.vector.tensor_scalar_max(iwih[:], t3[:], 0.0)
            # cls path: se = sum_j e via a split adder tree. The two big
            # levels run as bf16 TT adds on vector (2x-eligible, unlike
            # tensor_reduce); the three small levels + lse + ce hide on gpsimd.
            s1 = tmp(10 * TA, bf16)
            nc.vector.tensor_tensor(out=_v(s1[:], 0, [[10, TA], [1, 10]]),
                                    in0=_v(e[:], 0, [[NCLS, TA], [1, 10]]),
                                    in1=_v(e[:], 10, [[NCLS, TA], [1, 10]]),
                                    op=OP.add)
            s2 = tmp(5 * TA, bf16)
            nc.vector.tensor_tensor(out=_v(s2[:], 0, [[5, TA], [1, 5]]),
                                    in0=_v(s1[:], 0, [[10, TA], [1, 5]]),
                                    in1=_v(s1[:], 5, [[10, TA], [1, 5]]),
                                    op=OP.add)
            inter = tmp(TA)
            nc.vector.tensor_tensor(out=inter[:], in0=_v(iwih[:], 0, [[1, TA]]),
                                    in1=_v(iwih[:], TA, [[1, TA]]), op=OP.mult)
            # union side branch on gpsimd, in parallel with the inter chain
            areaA = tmp(TA)
            nc.gpsimd.tensor_tensor(out=areaA[:], in0=_v(wf[:], 0, [[1, TA]]),
                                    in1=_v(wf[:], TA, [[1, TA]]), op=OP.mult)
            areaA4 = tmp(TA)
            nc.gpsimd.tensor_scalar_mul(areaA4[:], areaA[:], 4.0)
            u1 = tmp(TA)
            nc.gpsimd.tensor_tensor(out=_v(u1[:], 0, [[A, T], [1, A]]),
                                    in0=_v(areaA4[:], 0, [[A, T], [1, A]]),
                                    in1=_v(fp[:], O["TAREA"], [[1, T], [0, A]]),
                                    op=OP.add)
            u2 = tmp(TA)
            nc.vector.tensor_tensor(out=u2[:], in0=u1[:], in1=inter[:],
                                    op=OP.subtract)
            rcp = tmp(TA)
            nc.vector.reciprocal(out=rcp[:], in_=u2[:])
            iou = tmp(TA)
            nc.vector.tensor_tensor(out=iou[:], in0=inter[:], in1=rcp[:],
                                    op=OP.mult)

            # first-argmax over a via bit-packed keys: clear the low 3
            # mantissa bits of iou (all ious >= 0, so int order == float
            # order) and OR in a per-anchor tiebreak (7 - a). Ties -- exact
            # at 8-ULP granularity -- resolve to the FIRST anchor, matching
            # jnp.argmax. Padding cells (iou all +0) one-hot-select anchor 0.
            ipk = tmp(TA, i32)
            nc.vector.tensor_scalar(out=ipk[:], in0=iou[:].bitcast(i32),
                                    scalar1=-8, scalar2=None,
                                    op0=OP.bitwise_and)
            ipk2 = tmp(TA, i32)
            nc.vector.tensor_tensor(out=_v(ipk2[:], 0, [[A, T], [1, A]]),
                                    in0=_v(ipk[:], 0, [[A, T], [1, A]]),
                                    in1=_v(fp[:], O["WCONST"],
                                           [[0, T], [1, A]]).bitcast(i32),
                                    op=OP.bitwise_or)
            rmax = tmp(T, i32)
            nc.vector.tensor_reduce(out=rmax[:],
                                    in_=_v(ipk2[:], 0, [[A, T], [1, A]]),
                                    axis=AX.X, op=OP.max)
            fmask = tmp(TA)
            nc.vector.tensor_tensor(out=_v(fmask[:], 0, [[A, T], [1, A]]),
                                    in0=_v(ipk2[:], 0, [[A, T], [1, A]]),
                                    in1=_v(rmax[:], 0, [[1, T], [0, A]]),
                                    op=OP.is_equal)
            s3 = tmp(2 * TA)
            nc.gpsimd.tensor_tensor(out=_v(s3[:], 0, [[2, TA], [1, 2]]),
                                    in0=_v(s2[:], 0, [[5, TA], [1, 2]]),
                                    in1=_v(s2[:], 2, [[5, TA], [1, 2]]),
                                    op=OP.add)
            s4 = tmp(TA)
            nc.gpsimd.tensor_tensor(out=_v(s4[:], 0, [[1, TA]]),
                                    in0=_v(s3[:], 0, [[2, TA]]),
                                    in1=_v(s3[:], 1, [[2, TA]]),
                                    op=OP.add)
            se = tmp(TA)
            nc.gpsimd.tensor_tensor(out=_v(se[:], 0, [[1, TA]]),
                                    in0=_v(s4[:], 0, [[1, TA]]),
                                    in1=_v(s2[:], 4, [[5, TA]]),
                                    op=OP.add)
            lgf = tmp(TA)
            nc.gpsimd.tensor_copy(out=lgf[:], in_=se[:].bitcast(i32))
            lse = tmp(TA)
            nc.gpsimd.tensor_scalar(out=lse[:], in0=lgf[:],
                                    scalar1=LN2 / (1 << 23),
                                    scalar2=-LOG_BIAS * LN2,
                                    op0=OP.mult, op1=OP.add)
            nc.gpsimd.tensor_tensor(out=_v(SRC[:], 6 * TA, [[A, T], [1, A]]),
                                    in0=_v(lse[:], 0, [[A, T], [1, A]]),
                                    in1=_v(fpb[:], O["SAUX"], [[A, T], [1, A]]),
                                    op=OP.subtract)

            # best-anchor selection of all seven quantities in one mul+reduce
            selm = pool.tile([P, 7 * TA], f32, name="selm")
            nc.vector.tensor_tensor(out=_v(selm[:], 0, [[TA, 7], [1, TA]]),
                                    in0=_v(SRC[:], 0, [[TA, 7], [1, TA]]),
                                    in1=_v(fmask[:], 0, [[0, 7], [1, TA]]),
                                    op=OP.mult)
            FIN = pool.tile([P, 7 * T], f32, name="FIN")
            nc.vector.tensor_reduce(out=_v(FIN[:], 0, [[T, 7], [1, T]]),
                                    in_=_v(selm[:], 0, [[TA, 7], [A, T], [1, A]]),
                                    axis=AX.X, op=OP.add)

            # box: FIN q0..3 -> oscl4*(sel - aux4)^2, written back into q0..3
            d4 = tmp(4 * T)
            nc.vector.tensor_tensor(out=d4[:], in0=_v(FIN[:], 0, [[1, 4 * T]]),
                                    in1=_v(fpb[:], O["AUX4"], [[1, 4 * T]]),
                                    op=OP.subtract)
            nc.vector.tensor_tensor(out=_v(FIN[:], 0, [[1, 4 * T]]),
                                    in0=d4[:], in1=d4[:], op=OP.mult)

            # one grouped reduce -> partials cols 0..6
            nc.vector.tensor_reduce(out=_v(partials[:], 0, [[1, 7]]),
                                    in_=_v(FIN[:], 0, [[T, 7], [1, T]]),
                                    axis=AX.X, op=OP.add)

            nc.sync.dma_start(out=partials_d[:], in_=partials[:])

    if split:
        _split_multi_waits(nc)
    return nc


# -------------------------------------------------------------- shard builder
def _make_in_maps(out, gt_boxes, anchor_np, gt_classes_np, num_box_np, T):
    obj, xo, yo, tw, th, cls_t = _build_target_np(gt_boxes, gt_classes_np,
                                                  num_box_np)
    SLOTS = P * T
    TA = T * A
    out_r = out.reshape(B, A, 25, HWC)
    sqa = np.sqrt(anchor_np)                       # [A, 2]

    in_maps = []
    for c in range(CORES):
        sl = slice(c * BC, (c + 1) * BC)
        ob = obj[sl]                               # [BC, HWC]
        bloc, hwloc = np.nonzero(ob > 0)
        K = len(bloc)
        assert K <= SLOTS

        def place(vals):
            buf = np.zeros(SLOTS, dtype=np.float32)
            buf[:K] = vals
            return buf.reshape(P, T)

        objv = place(np.ones(K, dtype=np.float32))
        xov = place(xo[sl][bloc, hwloc])
        yov = place(yo[sl][bloc, hwloc])
        twv = place(tw[sl][bloc, hwloc])
        thv = place(th[sl][bloc, hwloc])

        # occupied-cell prediction channels [K, A, 25]
        colsb = np.zeros((SLOTS, A, 25), dtype=np.float32)
        if K:
            colsb[:K] = out_r[sl].transpose(0, 3, 1, 2)[bloc, hwloc]
        # cols_xw (t, ch{x,y,conf,w,h}, a)
        sel = colsb[:, :, [21, 22, 20, 23, 24]]            # (t, a, ch)
        cols_xw = np.ascontiguousarray(
            sel.reshape(P, T, A, 5).transpose(0, 1, 3, 2)).reshape(P, 25 * T)
        logits = np.ascontiguousarray(
            colsb[:, :, :20]).reshape(P, 100 * T)          # (t, a, j)

        # target-class logit per (t, a); padding slots get the exact device
        # lse of se=20.0 (logits 0 -> e all 1) so padded ce comes out 0
        clsv = place(cls_t[sl][bloc, hwloc].astype(np.float32)).astype(np.int64)
        s_aux = np.take_along_axis(
            colsb[:, :, :20].reshape(SLOTS, A, 20),
            clsv.reshape(SLOTS, 1, 1).repeat(A, axis=1), axis=2
        )[:, :, 0].astype(np.float32)
        i32pad = float(np.float32(20.0).view(np.int32))
        lse_pad = np.float32(np.float32(i32pad * np.float32(LN2 / (1 << 23)))
                             + np.float32(-LOG_BIAS * LN2))
        s_aux[K:] = lse_pad
        s_aux = s_aux.reshape(P, TA)

        # xi-space target box edges (d{x,y}, t): center 2o-1, half-width t_wh
        cxv = 2.0 * xov - 1.0
        cyv = 2.0 * yov - 1.0
        b1 = np.stack([cxv - twv, cyv - thv], axis=1).reshape(P, 2 * T)
        b2 = np.stack([cxv + twv, cyv + thv], axis=1).reshape(P, 2 * T)
        tarea = (4.0 * twv * thv).reshape(P, T)

        # argmax tiebreak bits (7 - a) as raw int32, shipped bitcast as f32
        wconst = np.broadcast_to(
            (7 - np.arange(A, dtype=np.int32)).view(np.float32), (P, A))
        wconst = np.concatenate(
            [wconst, np.zeros((P, 5 * T - A), dtype=np.float32)], axis=1)
        sqav = np.broadcast_to(sqa.reshape(1, 10), (P, 10))

        # AUX4 in q-order (w, h, x, y); padding rows get the exact padded
        # prediction (sqrt(anchor0), tanh(0)=0) so their sq-diffs vanish
        aux4 = np.stack([np.sqrt(twv), np.sqrt(thv), cxv, cyv], axis=1)
        a4f = aux4.transpose(0, 2, 1).reshape(SLOTS, 4)
        a4f[K:] = [float(sqa[0, 0]), float(sqa[0, 1]), 0.0, 0.0]
        aux4 = np.ascontiguousarray(
            a4f.reshape(P, T, 4).transpose(0, 2, 1)).reshape(P, 4 * T)

        fpackA = np.concatenate(
            [cols_xw, b1, b2, tarea, wconst, sqav], axis=1)
        fpackB = np.concatenate([s_aux, aux4], axis=1)

        # dense conf channels: [BC, A, HWC] -> [P, 1280] bf16
        confd = out_r[sl][:, :, 20, :].reshape(P, -1)

        in_maps.append({
            "fpackA": np.ascontiguousarray(fpackA, dtype=np.float32),
            "fpackB": np.ascontiguousarray(fpackB, dtype=np.float32),
            "lgpack": _bf16(logits),
            "confd": _bf16(confd),
        })
    return in_maps


# ---------------------------------------------------------------- entry point
def kernel(out, gt_boxes, anchor, gt_classes, num_box):
    from concourse.bass_utils import run_bass_kernel_spmd

    out = np.ascontiguousarray(np.asarray(out, dtype=np.float32))
    gt_boxes = np.asarray(gt_boxes, dtype=np.float32)
    anchor_np = np.asarray(anchor, dtype=np.float32)
    gt_classes_np = np.asarray(gt_classes)
    num_box_np = np.asarray(num_box)

    # per-core occupied-cell counts decide the compiled tile factor T
    obj = _build_target_np(gt_boxes, gt_classes_np, num_box_np)[0]
    ks = [int((obj[c * BC:(c + 1) * BC] > 0).sum()) for c in range(CORES)]
    maxk = max(ks)
    T = max(1, -(-maxk // P))
    assert maxk <= 13 * P and T <= 13

    in_maps = _make_in_maps(out, gt_boxes, anchor_np, gt_classes_np,
                            num_box_np, T)

    import os
    key = f"nc{T}"
    if key not in _CACHE:
        _CACHE[key] = _build_nc(T)
    trace = os.environ.get("KERNEL_TRACE", "0") == "1"
    res = None
    for attempt in range(3):
        try:
            res = run_bass_kernel_spmd(_CACHE[key], in_maps,
                                       core_ids=list(range(CORES)), trace=trace)
            break
        except Exception:
            # transient device/runtime hiccups (e.g. NRT_EXEC_UNIT_UNRECOVERABLE)
            # recover on retry; re-raise only if persistent
            if attempt == 2:
                raise
            import time
            time.sleep(2.0)
    if trace:
        print(f"HW exec time: {res.exec_time_ns} ns  (mean {res.mean_exec_time_ns})")

    cols = np.zeros(12, dtype=np.float64)
    for c in range(CORES):
        cols += res.results[c]["partials"].astype(np.float64).sum(axis=0)
    K = float(sum(ks))
    box_loss = np.float32(LAM_COORD / B * (cols[0] + cols[1]
                                            + 0.25 * (cols[2] + cols[3])))
    conf_loss = np.float32(LAM_OBJ / B * 0.25 * (cols[5] - 2.0 * cols[4] + K))
    nob_c = 0.25 * (cols[5] + 2.0 * cols[4] + K)
    dense = 0.25 * (float(B * A * HWC) + 2.0 * cols[7] + cols[8])
    noobj_loss = np.float32(LAM_NOOBJ / B * (dense - nob_c))
    cls_loss = np.float32(LAM_CLS / B * cols[6])
    return (box_loss, conf_loss, noobj_loss, cls_loss)
